# revision 14
# baseline (speedup 1.0000x reference)
"""Multi-head attention (B=4, S=2048, D=1024, H=16) on 8 TRN2 NeuronCores.

Sharding: data-parallel over batch (4) x tensor-parallel over head halves (2).
Core c handles batch b = c//2 and heads [8g, 8g+8) where g = c%2.
Each core computes a partial [S, D] output-projection contribution; the host
sums the two head-group partials per batch.

All activations are passed to the device pre-transposed (feature dim on
partitions) so the kernel needs no on-device transposes:
  - projections contract over d (model dim) with host-transposed x^T,
  - scores are built transposed [k, q] so exp() output feeds the P @ V
    matmul directly as the moving operand,
  - P @ [V | 1] yields the softmax denominator as row 64 of U^T for free,
  - normalized U^T tiles are exactly the stationary layout w_o needs.

Matmul operands are bf16 (fp32 PSUM accumulation); fp32 moving operands
stream at half rate on TRN2, bf16 at full rate. Head pairs share the PE
array via row tiling (partitions 0-63 / 64-127) so the DK=64 score matmuls
run concurrently.

The scalar engine's exp() stream (~272us busy) is the critical path, and
its input buffer is only 3 PSUM score tiles (~3.4us of lookahead), so all
non-attention PE work (K/V/Q projection accumulations, w_o output
projection) is chopped into single-accumulator "pieces" and injected at
most two at a time between attention turns. Attention itself runs
kp-major: for each k-range, all four head pairs take a turn, giving 16
uniform injection slots per q-chunk.

PSUM budget (8 banks): 3 x [128,1024] score tiles (6 banks) + one shared
2-slot pool (2 banks) for every [<=128,512] accumulator; attention U
accumulates in SBUF via DVE adds of 4-k-tile PSUM partials so no PSUM slot
is held for long.
"""

import numpy as np

B, S, D, H = 4, 2048, 1024, 16
DK = D // H          # 64
G = 2                # head groups (tensor-parallel degree per batch)
HL = H // G          # 8 local heads per core
DV = HL * DK         # 512 local value dim
N_CORES = 8

_cached = {}


def _build():
    import concourse.bass as bass
    import concourse.tile as tile
    from concourse import bacc, mybir

    f32 = mybir.dt.float32
    bf16 = mybir.dt.bfloat16
    EXP = mybir.ActivationFunctionType.Exp

    nc = bacc.Bacc("TRN2", target_bir_lowering=False, debug=False,
                   num_devices=N_CORES)

    xqT = nc.dram_tensor("xqT", [D, S], bf16, kind="ExternalInput").ap()
    xkT = nc.dram_tensor("xkT", [D, S], bf16, kind="ExternalInput").ap()
    xvT = nc.dram_tensor("xvT", [D, S], bf16, kind="ExternalInput").ap()
    wqT = nc.dram_tensor("wqT", [D, DV], bf16, kind="ExternalInput").ap()
    wkT = nc.dram_tensor("wkT", [D, DV], bf16, kind="ExternalInput").ap()
    wvT = nc.dram_tensor("wvT", [D, DV], bf16, kind="ExternalInput").ap()
    woT = nc.dram_tensor("woT", [DV, D], bf16, kind="ExternalInput").ap()
    out = nc.dram_tensor("out", [S, D], f32, kind="ExternalOutput").ap()

    ND = D // 128     # 8 d-tiles
    NS = S // 128     # 16 s-tiles (k-tiles)
    NQC = S // 512    # 4 q-chunks
    NT = DV // 128    # 4 dk/dv-tiles
    NHP = HL // 2     # 4 head pairs

    with tile.TileContext(nc) as tc:
        with (
            tc.tile_pool(name="persist", bufs=1) as persist,
            tc.tile_pool(name="stage", bufs=32) as stage,
            tc.tile_pool(name="wpool", bufs=8) as wpool,
            tc.tile_pool(name="spool", bufs=3, space=bass.MemorySpace.PSUM) as spool,
            tc.tile_pool(name="upool", bufs=2, space=bass.MemorySpace.PSUM) as upool,
            tc.tile_pool(name="ppool", bufs=4) as ppool,
            tc.tile_pool(name="rpool", bufs=3) as rpool,
            tc.tile_pool(name="obuf", bufs=3) as obuf,
        ):
            QT = {}    # [t][qc] -> [128, 512] tiles of Q^T (dk rows, q cols)
            KT = {}    # [t][c]  -> [128, 512]
            Vaug = {}  # [kt] -> [128, 8, 65]: per-head V columns + ones col
            outT = {}  # [qc][t] -> [128, 512] normalized attention out^T
            wks, wvs, wqs, wos = [], [], [], []
            st_ = {}   # per (qc, hp) attention state
            xq_stage = {}

            def uacc(shape):
                return upool.tile(shape, f32, tag="u", name="uacc")

            def emit_w_loads():
                for lst, name, src in ((wks, "wk", wkT), (wvs, "wv", wvT),
                                       (wqs, "wq", wqT)):
                    for d in range(ND):
                        wt = wpool.tile([128, DV], bf16, tag="w", name=name,
                                        bufs=24)
                        nc.sync.dma_start(wt[:], src[128 * d:128 * (d + 1), :])
                        lst.append(wt)
                for t in range(NT):
                    wo = wpool.tile([128, D], bf16, tag=f"wo{t}", name="wo",
                                    bufs=1)
                    nc.sync.dma_start(wo[:], woT[128 * t:128 * (t + 1), :])
                    wos.append(wo)

            def emit_kv_dmas(c):
                xks, xvs = [], []
                for d in range(ND):
                    xt = stage.tile([128, 512], bf16, tag="act", name="xk")
                    nc.sync.dma_start(
                        xt[:], xkT[128 * d:128 * (d + 1), 512 * c:512 * (c + 1)])
                    xks.append(xt)
                for d in range(ND):
                    xt = stage.tile([128, 512], bf16, tag="act", name="xv")
                    nc.sync.dma_start(
                        xt[:], xvT[128 * d:128 * (d + 1), 512 * c:512 * (c + 1)])
                    xvs.append(xt)
                return xks, xvs

            def piece_kproj(xks, c, t):
                def go():
                    acc = uacc([128, 512])
                    for d in range(ND):
                        nc.tensor.matmul(
                            acc[:], wks[d][:, 128 * t:128 * (t + 1)], xks[d][:],
                            start=(d == 0), stop=(d == ND - 1))
                    dt_ = persist.tile([128, 512], bf16, tag=f"kT{t}_{c}",
                                       name="kT")
                    nc.vector.tensor_copy(dt_[:], acc[:])
                    KT.setdefault(t, {})[c] = dt_
                return go

            def piece_vproj(xvs, c, ktl):
                def go():
                    kt = 4 * c + ktl
                    acc = uacc([128, 512])
                    for d in range(ND):
                        nc.tensor.matmul(
                            acc[:], xvs[d][:, 128 * ktl:128 * (ktl + 1)],
                            wvs[d][:],
                            start=(d == 0), stop=(d == ND - 1))
                    va = persist.tile([128, HL, DK + 1], bf16, tag=f"vaug{kt}",
                                      name="vaug")
                    nc.vector.tensor_copy(
                        va[:, :, 0:DK],
                        acc[:].rearrange("p (h k) -> p h k", h=HL))
                    nc.vector.tensor_copy(
                        va[:, :, DK], nc.const_aps.tensor(1.0, (128, HL), bf16))
                    Vaug[kt] = va
                return go

            def emit_xq_dmas(qc):
                xs = []
                for d in range(ND):
                    xt = stage.tile([128, 512], bf16, tag="act", name="xq")
                    nc.sync.dma_start(
                        xt[:], xqT[128 * d:128 * (d + 1), 512 * qc:512 * (qc + 1)])
                    xs.append(xt)
                xq_stage[qc] = xs

            def piece_qproj(qc, t):
                def go():
                    xs = xq_stage[qc]
                    acc = uacc([128, 512])
                    for d in range(ND):
                        nc.tensor.matmul(
                            acc[:], wqs[d][:, 128 * t:128 * (t + 1)], xs[d][:],
                            start=(d == 0), stop=(d == ND - 1))
                    dt_ = persist.tile([128, 512], bf16, tag=f"qT{t}_{qc}",
                                       name="qT")
                    nc.vector.tensor_copy(dt_[:], acc[:])
                    QT.setdefault(t, {})[qc] = dt_
                return go

            def piece_wo(qc, st, ncol):
                # final[s, n] = sum_dv outT[dv, s] * woT[dv, n]
                def go():
                    acc = uacc([128, 512])
                    for t in range(NT):
                        nc.tensor.matmul(
                            acc[:],
                            outT[qc][t][:, 128 * st:128 * (st + 1)],
                            wos[t][:, 512 * ncol:512 * (ncol + 1)],
                            start=(t == 0), stop=(t == NT - 1))
                    ob = obuf.tile([128, 512], f32, tag="ob", name="ob")
                    nc.vector.tensor_copy(ob[:], acc[:])
                    nc.sync.dma_start(
                        out[512 * qc + 128 * st:512 * qc + 128 * (st + 1),
                            512 * ncol:512 * (ncol + 1)],
                        ob[:])
                return go

            def emit_attn_turn(qc, hp, c):
                # head pair (2hp, 2hp+1) = partition halves of tile hp: their
                # DK=64 score matmuls row-tile the PE array (rows 0-63/64-127).
                # One turn covers k-tiles [4c, 4c+4) = one PSUM partial group,
                # folded into the SBUF accumulator Usb.
                t = hp
                s = st_.setdefault((qc, hp), {})
                if c == 0:
                    s["Usb"] = [rpool.tile([65, 512], f32, tag=f"usb{hp}_{i}",
                                           name="usb", bufs=2)
                                for i in range(2)]
                Up = [None, None]
                for kp in (2 * c, 2 * c + 1):
                    sc = [spool.tile([128, 1024], f32, tag="sc", name="sc")
                          for _ in range(2)]
                    for j in range(2):
                        kt = 2 * kp + j
                        for i in range(2):
                            po = 64 * i
                            nc.tensor.matmul(
                                sc[i][:, 512 * j:512 * (j + 1)],
                                KT[t][kt // 4][po:po + 64,
                                               128 * (kt % 4):128 * (kt % 4 + 1)],
                                QT[t][qc][po:po + 64, :],
                                start=True, stop=True)
                    for i in range(2):
                        P = ppool.tile([128, 1024], bf16, tag="p", name="p")
                        nc.scalar.activation(P[:], sc[i][:], EXP, scale=0.125)
                        if kp % 2 == 0:
                            Up[i] = uacc([65, 512])
                        for j in range(2):
                            kt = 2 * kp + j
                            nc.tensor.matmul(
                                Up[i][:],
                                Vaug[kt][:, 2 * hp + i, :],
                                P[:, 512 * j:512 * (j + 1)],
                                start=(kt % 4 == 0), stop=(kt % 4 == 3))
                        if kp % 2 == 1:
                            if c == 0:
                                nc.vector.tensor_copy(s["Usb"][i][:], Up[i][:])
                            else:
                                nc.vector.tensor_add(s["Usb"][i][:],
                                                     s["Usb"][i][:], Up[i][:])

            def emit_normalize(qc, hp):
                # rows 0..63 of U divided by row 64 (the ones-column sum),
                # written into out^T. Engine ops keep operands on one
                # partition range; cross-partition moves via SBUF-SBUF DMA.
                t = hp
                Usb = st_[(qc, hp)]["Usb"]
                ot = persist.tile([128, 512], bf16, tag=f"oT{t}_{qc % 2}",
                                  name="oT")
                outT.setdefault(qc, {})[t] = ot
                for i in range(2):
                    rrow = rpool.tile([1, 512], f32, tag="rrow", name="rrow")
                    nc.sync.dma_start(rrow[:], Usb[i][64:65, :])
                    rrec = rpool.tile([1, 512], f32, tag="rrec", name="rrec")
                    nc.vector.reciprocal_approx_fast(rrec[:], rrow[:])
                    rb = rpool.tile([64, 512], f32, tag="rb", name="rb")
                    nc.gpsimd.partition_broadcast(rb[:], rrec[:])
                    if i == 0:
                        nc.vector.tensor_mul(ot[0:64, :], Usb[i][0:64, :],
                                             rb[:])
                    else:
                        stg = rpool.tile([64, 512], bf16, tag="stg",
                                         name="stg")
                        nc.vector.tensor_mul(stg[:], Usb[i][0:64, :], rb[:])
                        nc.sync.dma_start(ot[64:128, :], stg[:])

            # ---- emission schedule ----
            # slot_map[qc][(c, hp)] = pieces emitted just BEFORE that turn;
            # placement keeps every non-attention chain within ~2 accs of an
            # attention turn so neither PE nor the scalar engine starves.
            emit_w_loads()
            xks0, xvs0 = emit_kv_dmas(0)
            emit_xq_dmas(0)
            kvx = {0: (xks0, xvs0)}
            for c2 in range(1, 4):
                kvx[c2] = emit_kv_dmas(c2)
            for qc in range(1, NQC):
                emit_xq_dmas(qc)

            # prologue: just enough for the first attention turn
            for ktl in range(4):
                piece_vproj(xvs0, 0, ktl)()
            piece_kproj(xks0, 0, 0)()
            piece_qproj(0, 0)()

            slot_map = {qc: {} for qc in range(NQC)}

            def put(qc, c, hp, piece):
                slot_map[qc].setdefault((c, hp), []).append(piece)

            # qc0: remaining K/V chunks + rest of chunk-0/Q0 projections
            for c2 in range(1, 4):
                xks, xvs = kvx[c2]
                put(0, c2 - 1, 2, piece_vproj(xvs, c2, 0))
                put(0, c2 - 1, 2, piece_vproj(xvs, c2, 1))
                put(0, c2 - 1, 3, piece_vproj(xvs, c2, 2))
                put(0, c2 - 1, 3, piece_vproj(xvs, c2, 3))
                for t in range(NT):
                    put(0, c2, t, piece_kproj(xks, c2, t))
            for t in range(1, NT):
                put(0, 0, t, piece_kproj(xks0, 0, t))
                put(0, 0, t, piece_qproj(0, t))
            for t in range(NT):
                put(0, 3, t, piece_qproj(1, t))
            # qc1/qc2: next Q projection early, previous w_o spread mid-chunk
            for qc in (1, 2):
                if qc + 1 < NQC:
                    for t in range(NT):
                        put(qc, 0, t, piece_qproj(qc + 1, t))
                for idx in range(8):
                    st2, ncol = divmod(idx, 2)
                    put(qc, 1 + idx // 4, idx % 4, piece_wo(qc - 1, st2, ncol))
            for idx in range(8):
                st2, ncol = divmod(idx, 2)
                put(3, idx // 4, idx % 4, piece_wo(2, st2, ncol))

            for qc in range(NQC):
                for c in range(4):
                    for hp in range(NHP):
                        for piece in slot_map[qc].get((c, hp), ()):
                            piece()
                        emit_attn_turn(qc, hp, c)
                        if c == 3:
                            emit_normalize(qc, hp)
            for st2 in range(4):
                for ncol in range(2):
                    piece_wo(NQC - 1, st2, ncol)()

    nc.compile()
    return nc


def kernel(query, key, value, w_q, w_k, w_v, w_o):
    import ml_dtypes
    from concourse.bass_utils import run_bass_kernel_spmd

    if "nc" not in _cached:
        _cached["nc"] = _build()
    nc = _cached["nc"]

    bf = ml_dtypes.bfloat16
    query = np.asarray(query, dtype=np.float32)
    key = np.asarray(key, dtype=np.float32)
    value = np.asarray(value, dtype=np.float32)
    w_q = np.asarray(w_q, dtype=np.float32)
    w_k = np.asarray(w_k, dtype=np.float32)
    w_v = np.asarray(w_v, dtype=np.float32)
    w_o = np.asarray(w_o, dtype=np.float32)

    def c(a):
        return np.ascontiguousarray(a).astype(bf)

    in_maps = []
    for core in range(N_CORES):
        b, g = core // G, core % G
        rows = slice(DV * g, DV * (g + 1))
        in_maps.append({
            "xqT": c(query[b].T),
            "xkT": c(key[b].T),
            "xvT": c(value[b].T),
            "wqT": c(w_q[rows, :].T),
            "wkT": c(w_k[rows, :].T),
            "wvT": c(w_v[rows, :].T),
            "woT": c(w_o[:, rows].T),
        })

    res = run_bass_kernel_spmd(nc, in_maps, list(range(N_CORES)))
    full = np.empty((B, S, D), np.float32)
    for b in range(B):
        full[b] = res.results[G * b]["out"] + res.results[G * b + 1]["out"]
    return full


# revision 17
# speedup vs baseline: 1.0160x; 1.0160x over previous
"""Multi-head attention (B=4, S=2048, D=1024, H=16) on 8 TRN2 NeuronCores.

Sharding: data-parallel over batch (4) x tensor-parallel over head halves (2).
Core c handles batch b = c//2 and heads [8g, 8g+8) where g = c%2.
Each core computes a partial [S, D] output-projection contribution; the host
sums the two head-group partials per batch.

All activations are passed to the device pre-transposed (feature dim on
partitions) so the kernel needs no on-device transposes:
  - projections contract over d (model dim) with host-transposed x^T,
  - scores are built transposed [k, q] so exp() output feeds the P @ V
    matmul directly as the moving operand,
  - P @ [V | 1] yields the softmax denominator as row 64 of U^T for free,
  - normalized U^T tiles are exactly the stationary layout w_o needs.

Matmul operands are bf16 (fp32 PSUM accumulation); fp32 moving operands
stream at half rate on TRN2, bf16 at full rate. Head pairs share the PE
array via row tiling (partitions 0-63 / 64-127) so the DK=64 score matmuls
run concurrently.

The scalar engine's exp() stream (~272us busy) is the critical path, and
its input buffer is only 3 PSUM score tiles (~3.4us of lookahead), so all
non-attention PE work (K/V/Q projection accumulations, w_o output
projection) is chopped into single-accumulator "pieces" and injected at
most two at a time between attention turns. Attention itself runs
kp-major: for each k-range, all four head pairs take a turn, giving 16
uniform injection slots per q-chunk.

PSUM budget (8 banks): 3 x [128,1024] score tiles (6 banks) + one shared
2-slot pool (2 banks) for every [<=128,512] accumulator; attention U
accumulates in SBUF via DVE adds of 4-k-tile PSUM partials so no PSUM slot
is held for long.
"""

import numpy as np

B, S, D, H = 4, 2048, 1024, 16
DK = D // H          # 64
G = 2                # head groups (tensor-parallel degree per batch)
HL = H // G          # 8 local heads per core
DV = HL * DK         # 512 local value dim
N_CORES = 8

_cached = {}


def _build():
    import concourse.bass as bass
    import concourse.tile as tile
    from concourse import bacc, mybir

    f32 = mybir.dt.float32
    bf16 = mybir.dt.bfloat16
    EXP = mybir.ActivationFunctionType.Exp

    nc = bacc.Bacc("TRN2", target_bir_lowering=False, debug=False,
                   num_devices=N_CORES)

    xqT = nc.dram_tensor("xqT", [D, S], bf16, kind="ExternalInput").ap()
    xkT = nc.dram_tensor("xkT", [D, S], bf16, kind="ExternalInput").ap()
    xvT = nc.dram_tensor("xvT", [D, S], bf16, kind="ExternalInput").ap()
    wqT = nc.dram_tensor("wqT", [D, DV], bf16, kind="ExternalInput").ap()
    wkT = nc.dram_tensor("wkT", [D, DV], bf16, kind="ExternalInput").ap()
    wvT = nc.dram_tensor("wvT", [D, DV], bf16, kind="ExternalInput").ap()
    woT = nc.dram_tensor("woT", [DV, D], bf16, kind="ExternalInput").ap()
    out = nc.dram_tensor("out", [S, D], f32, kind="ExternalOutput").ap()

    ND = D // 128     # 8 d-tiles
    NS = S // 128     # 16 s-tiles (k-tiles)
    NQC = S // 512    # 4 q-chunks
    NT = DV // 128    # 4 dk/dv-tiles
    NHP = HL // 2     # 4 head pairs

    with tile.TileContext(nc) as tc:
        with (
            tc.tile_pool(name="persist", bufs=1) as persist,
            tc.tile_pool(name="stage", bufs=32) as stage,
            tc.tile_pool(name="wpool", bufs=8) as wpool,
            tc.tile_pool(name="spool", bufs=3, space=bass.MemorySpace.PSUM) as spool,
            tc.tile_pool(name="upool", bufs=2, space=bass.MemorySpace.PSUM) as upool,
            tc.tile_pool(name="ppool", bufs=4) as ppool,
            tc.tile_pool(name="rpool", bufs=3) as rpool,
            tc.tile_pool(name="obuf", bufs=3) as obuf,
        ):
            QT = {}    # [t][qc] -> [128, 512] tiles of Q^T (dk rows, q cols)
            KT = {}    # [t][c]  -> [128, 512]
            Vaug = {}  # [kt] -> [128, 8, 65]: per-head V columns + ones col
            outT = {}  # [qc][t] -> [128, 512] normalized attention out^T
            wks, wvs, wqs, wos = [], [], [], []
            st_ = {}   # per (qc, hp) attention state
            xq_stage = {}

            def uacc(shape):
                return upool.tile(shape, f32, tag="u", name="uacc")

            def emit_w_loads(lst, name, src):
                for d in range(ND):
                    wt = wpool.tile([128, DV], bf16, tag="w", name=name,
                                    bufs=24)
                    nc.sync.dma_start(wt[:], src[128 * d:128 * (d + 1), :])
                    lst.append(wt)

            def emit_wo_loads():
                for t in range(NT):
                    wo = wpool.tile([128, D], bf16, tag=f"wo{t}", name="wo",
                                    bufs=1)
                    nc.sync.dma_start(wo[:], woT[128 * t:128 * (t + 1), :])
                    wos.append(wo)

            def emit_x_dmas(src, c, name):
                xs = []
                for d in range(ND):
                    xt = stage.tile([128, 512], bf16, tag="act", name=name)
                    nc.sync.dma_start(
                        xt[:], src[128 * d:128 * (d + 1), 512 * c:512 * (c + 1)])
                    xs.append(xt)
                return xs

            def piece_kproj(xks, c, t):
                def go():
                    acc = uacc([128, 512])
                    for d in range(ND):
                        nc.tensor.matmul(
                            acc[:], wks[d][:, 128 * t:128 * (t + 1)], xks[d][:],
                            start=(d == 0), stop=(d == ND - 1))
                    dt_ = persist.tile([128, 512], bf16, tag=f"kT{t}_{c}",
                                       name="kT")
                    nc.vector.tensor_copy(dt_[:], acc[:])
                    KT.setdefault(t, {})[c] = dt_
                return go

            def piece_vproj(xvs, c, ktl):
                def go():
                    kt = 4 * c + ktl
                    acc = uacc([128, 512])
                    for d in range(ND):
                        nc.tensor.matmul(
                            acc[:], xvs[d][:, 128 * ktl:128 * (ktl + 1)],
                            wvs[d][:],
                            start=(d == 0), stop=(d == ND - 1))
                    va = persist.tile([128, HL, DK + 1], bf16, tag=f"vaug{kt}",
                                      name="vaug")
                    nc.vector.tensor_copy(
                        va[:, :, 0:DK],
                        acc[:].rearrange("p (h k) -> p h k", h=HL))
                    nc.vector.tensor_copy(
                        va[:, :, DK], nc.const_aps.tensor(1.0, (128, HL), bf16))
                    Vaug[kt] = va
                return go

            def emit_xq_dmas(qc):
                xs = []
                for d in range(ND):
                    xt = stage.tile([128, 512], bf16, tag="act", name="xq")
                    nc.sync.dma_start(
                        xt[:], xqT[128 * d:128 * (d + 1), 512 * qc:512 * (qc + 1)])
                    xs.append(xt)
                xq_stage[qc] = xs

            def piece_qproj(qc, t):
                def go():
                    xs = xq_stage[qc]
                    acc = uacc([128, 512])
                    for d in range(ND):
                        nc.tensor.matmul(
                            acc[:], wqs[d][:, 128 * t:128 * (t + 1)], xs[d][:],
                            start=(d == 0), stop=(d == ND - 1))
                    dt_ = persist.tile([128, 512], bf16, tag=f"qT{t}_{qc}",
                                       name="qT")
                    nc.vector.tensor_copy(dt_[:], acc[:])
                    QT.setdefault(t, {})[qc] = dt_
                return go

            def piece_wo(qc, st, ncol):
                # final[s, n] = sum_dv outT[dv, s] * woT[dv, n]
                def go():
                    acc = uacc([128, 512])
                    for t in range(NT):
                        nc.tensor.matmul(
                            acc[:],
                            outT[qc][t][:, 128 * st:128 * (st + 1)],
                            wos[t][:, 512 * ncol:512 * (ncol + 1)],
                            start=(t == 0), stop=(t == NT - 1))
                    ob = obuf.tile([128, 512], f32, tag="ob", name="ob")
                    nc.vector.tensor_copy(ob[:], acc[:])
                    nc.sync.dma_start(
                        out[512 * qc + 128 * st:512 * qc + 128 * (st + 1),
                            512 * ncol:512 * (ncol + 1)],
                        ob[:])
                return go

            def emit_attn_turn(qc, hp, c):
                # head pair (2hp, 2hp+1) = partition halves of tile hp: their
                # DK=64 score matmuls row-tile the PE array (rows 0-63/64-127).
                # One turn covers k-tiles [4c, 4c+4) = one PSUM partial group,
                # folded into the SBUF accumulator Usb.
                t = hp
                s = st_.setdefault((qc, hp), {})
                if c == 0:
                    s["Usb"] = [rpool.tile([65, 512], f32, tag=f"usb{hp}_{i}",
                                           name="usb", bufs=2)
                                for i in range(2)]
                Up = [None, None]
                for kp in (2 * c, 2 * c + 1):
                    sc = [spool.tile([128, 1024], f32, tag="sc", name="sc")
                          for _ in range(2)]
                    for j in range(2):
                        kt = 2 * kp + j
                        for i in range(2):
                            po = 64 * i
                            nc.tensor.matmul(
                                sc[i][:, 512 * j:512 * (j + 1)],
                                KT[t][kt // 4][po:po + 64,
                                               128 * (kt % 4):128 * (kt % 4 + 1)],
                                QT[t][qc][po:po + 64, :],
                                start=True, stop=True)
                    for i in range(2):
                        P = ppool.tile([128, 1024], bf16, tag="p", name="p")
                        nc.scalar.activation(P[:], sc[i][:], EXP, scale=0.125)
                        if kp % 2 == 0:
                            Up[i] = uacc([65, 512])
                        for j in range(2):
                            kt = 2 * kp + j
                            nc.tensor.matmul(
                                Up[i][:],
                                Vaug[kt][:, 2 * hp + i, :],
                                P[:, 512 * j:512 * (j + 1)],
                                start=(kt % 4 == 0), stop=(kt % 4 == 3))
                        if kp % 2 == 1:
                            if c == 0:
                                nc.vector.tensor_copy(s["Usb"][i][:], Up[i][:])
                            else:
                                nc.vector.tensor_add(s["Usb"][i][:],
                                                     s["Usb"][i][:], Up[i][:])

            def emit_normalize(qc, hp):
                # rows 0..63 of U divided by row 64 (the ones-column sum),
                # written into out^T. Engine ops keep operands on one
                # partition range; cross-partition moves via SBUF-SBUF DMA.
                t = hp
                Usb = st_[(qc, hp)]["Usb"]
                ot = persist.tile([128, 512], bf16, tag=f"oT{t}_{qc % 2}",
                                  name="oT")
                outT.setdefault(qc, {})[t] = ot
                for i in range(2):
                    rrow = rpool.tile([1, 512], f32, tag="rrow", name="rrow")
                    nc.sync.dma_start(rrow[:], Usb[i][64:65, :])
                    rrec = rpool.tile([1, 512], f32, tag="rrec", name="rrec")
                    nc.vector.reciprocal_approx_fast(rrec[:], rrow[:])
                    rb = rpool.tile([64, 512], f32, tag="rb", name="rb")
                    nc.gpsimd.partition_broadcast(rb[:], rrec[:])
                    if i == 0:
                        nc.vector.tensor_mul(ot[0:64, :], Usb[i][0:64, :],
                                             rb[:])
                    else:
                        stg = rpool.tile([64, 512], bf16, tag="stg",
                                         name="stg")
                        nc.vector.tensor_mul(stg[:], Usb[i][0:64, :], rb[:])
                        nc.sync.dma_start(ot[64:128, :], stg[:])

            # ---- emission schedule ----
            # slot_map[qc][(c, hp)] = pieces emitted just BEFORE that turn;
            # placement keeps every non-attention chain within ~2 accs of an
            # attention turn so neither PE nor the scalar engine starves.
            # DMA order: the V-projection path loads first (the prologue's
            # first PE work), prefetches of later chunks come last.
            emit_w_loads(wvs, "wv", wvT)
            xvs0 = emit_x_dmas(xvT, 0, "xv")
            emit_w_loads(wks, "wk", wkT)
            xks0 = emit_x_dmas(xkT, 0, "xk")
            emit_w_loads(wqs, "wq", wqT)
            emit_xq_dmas(0)
            kvx = {}
            for c2 in range(1, 4):
                kvx[c2] = (emit_x_dmas(xkT, c2, "xk"),
                           emit_x_dmas(xvT, c2, "xv"))
            for qc in range(1, NQC):
                emit_xq_dmas(qc)
            emit_wo_loads()

            # prologue: just enough for the first attention turn
            for ktl in range(4):
                piece_vproj(xvs0, 0, ktl)()
            piece_kproj(xks0, 0, 0)()
            piece_qproj(0, 0)()

            slot_map = {qc: {} for qc in range(NQC)}

            def put(qc, c, hp, piece):
                slot_map[qc].setdefault((c, hp), []).append(piece)

            # qc0: remaining K/V chunks + rest of chunk-0/Q0 projections
            for c2 in range(1, 4):
                xks, xvs = kvx[c2]
                put(0, c2 - 1, 2, piece_vproj(xvs, c2, 0))
                put(0, c2 - 1, 2, piece_vproj(xvs, c2, 1))
                put(0, c2 - 1, 3, piece_vproj(xvs, c2, 2))
                put(0, c2 - 1, 3, piece_vproj(xvs, c2, 3))
                for t in range(NT):
                    put(0, c2, t, piece_kproj(xks, c2, t))
            for t in range(1, NT):
                put(0, 0, t, piece_kproj(xks0, 0, t))
                put(0, 0, t, piece_qproj(0, t))
            for t in range(NT):
                put(0, 3, t, piece_qproj(1, t))
            # qc1/qc2: next Q projection early, previous w_o spread mid-chunk
            for qc in (1, 2):
                if qc + 1 < NQC:
                    for t in range(NT):
                        put(qc, 0, t, piece_qproj(qc + 1, t))
                for idx in range(8):
                    st2, ncol = divmod(idx, 2)
                    put(qc, 1 + idx // 4, idx % 4, piece_wo(qc - 1, st2, ncol))
            for idx in range(8):
                st2, ncol = divmod(idx, 2)
                put(3, idx // 4, idx % 4, piece_wo(2, st2, ncol))

            for qc in range(NQC):
                for c in range(4):
                    for hp in range(NHP):
                        for piece in slot_map[qc].get((c, hp), ()):
                            piece()
                        emit_attn_turn(qc, hp, c)
                        if c == 3:
                            emit_normalize(qc, hp)
            for st2 in range(4):
                for ncol in range(2):
                    piece_wo(NQC - 1, st2, ncol)()

    nc.compile()
    return nc


def kernel(query, key, value, w_q, w_k, w_v, w_o):
    import ml_dtypes
    from concourse.bass_utils import run_bass_kernel_spmd

    if "nc" not in _cached:
        _cached["nc"] = _build()
    nc = _cached["nc"]

    bf = ml_dtypes.bfloat16
    query = np.asarray(query, dtype=np.float32)
    key = np.asarray(key, dtype=np.float32)
    value = np.asarray(value, dtype=np.float32)
    w_q = np.asarray(w_q, dtype=np.float32)
    w_k = np.asarray(w_k, dtype=np.float32)
    w_v = np.asarray(w_v, dtype=np.float32)
    w_o = np.asarray(w_o, dtype=np.float32)

    def c(a):
        return np.ascontiguousarray(a).astype(bf)

    in_maps = []
    for core in range(N_CORES):
        b, g = core // G, core % G
        rows = slice(DV * g, DV * (g + 1))
        in_maps.append({
            "xqT": c(query[b].T),
            "xkT": c(key[b].T),
            "xvT": c(value[b].T),
            "wqT": c(w_q[rows, :].T),
            "wkT": c(w_k[rows, :].T),
            "wvT": c(w_v[rows, :].T),
            "woT": c(w_o[:, rows].T),
        })

    res = run_bass_kernel_spmd(nc, in_maps, list(range(N_CORES)))
    full = np.empty((B, S, D), np.float32)
    for b in range(B):
        full[b] = res.results[G * b]["out"] + res.results[G * b + 1]["out"]
    return full
